# revision 1
# baseline (speedup 1.0000x reference)
"""LDS (diagonal linear state space + AR) kernel for 8 Trainium2 cores.

Computation (per batch b):
    uB[t, s]   = sum_d x[t, d] * B[d, s]
    h[t]       = A * h[t-1] + uB[t]          (h[-1] = h0, A diagonal)
    lds[t, o]  = sum_s h[t, s] * C[s, o]
    out[t, o]  = sum_{i<10} sum_d M[o, d, i] * x[t-i, d]  +  lds[t+10, o]

Sharding: data-parallel over batch, 2 batches per core, no collectives.

On-chip layout is [feature, time]:
  - x is host-transposed/padded to xT [2, 2, 128, PAD+T] (b, d_chunk, d, t)
  - uB produced by f32r matmuls into PSUM [128s, 512t]
  - the recurrence runs as tensor_tensor_scan on VectorE, reading uB from
    PSUM and writing hT [128s, T+16] (tail zeroed for the +10 shift)
  - output tiles [128t, 256o] accumulate 8 C-matmuls + 20 AR matmuls in
    PSUM, then DMA straight to HBM (contiguous rows)
"""

import sys

if "/opt/trn_rl_repo" not in sys.path:
    sys.path.insert(0, "/opt/trn_rl_repo")

import numpy as np

import concourse.bass as bass
import concourse.mybir as mybir
from concourse import bacc
from concourse.tile import TileContext

BSZ = 16
SEQ = 2048
D = 256  # input dim
S = 1024  # state dim
O = 256  # output dim
KX = 10
N_CORES = 8
B_PER_CORE = BSZ // N_CORES  # 2

PAD = 16  # left zero-pad on time for the AR taps (needs >= KX-1 = 9)
HPAD = 16  # right zero-pad on h time for the +10 shift (needs >= KX)
TCH = 512  # uB matmul / scan chunk width (= 1 PSUM bank of fp32)
OTCH = 128  # output tile time width (= partition dim of out psum tile)

F32 = mybir.dt.float32
F32R = mybir.dt.float32r

_CACHED = {}


def _build_nc():
    nc = bass.Bass()

    xt_d = nc.dram_tensor("xt", [B_PER_CORE, 2, 128, PAD + SEQ], F32,
                          kind="ExternalInput")
    b_d = nc.dram_tensor("bmat", [2, 128, S], F32, kind="ExternalInput")
    c_d = nc.dram_tensor("cmat", [8, 128, O], F32, kind="ExternalInput")
    m_d = nc.dram_tensor("mmat", [KX, 2, 128, O], F32, kind="ExternalInput")
    ah_d = nc.dram_tensor("ah", [128, 16], F32, kind="ExternalInput")
    z_d = nc.dram_tensor("zt", [128, HPAD], F32, kind="ExternalInput")
    out_d = nc.dram_tensor("out", [B_PER_CORE, SEQ, O], F32,
                           kind="ExternalOutput")

    with TileContext(nc) as tc:
        with tc.tile_pool(name="persist", bufs=1) as persist, \
             tc.tile_pool(name="ht", bufs=10) as ht_pool, \
             tc.tile_pool(name="outsb", bufs=6) as out_sbuf, \
             tc.tile_pool(name="ubps", bufs=4, space="PSUM") as ub_psum, \
             tc.tile_pool(name="outps", bufs=4, space="PSUM") as out_psum:

            # ---- load persistent operands ----
            xt = {}
            for b in range(B_PER_CORE):
                for dch in range(2):
                    t = persist.tile([128, PAD + SEQ], F32R, tag=f"xt{b}{dch}")
                    nc.sync.dma_start(out=t[:], in_=xt_d[b, dch].bitcast(F32R))
                    xt[b, dch] = t
            bmat = {}
            for dch in range(2):
                t = persist.tile([128, S], F32R, tag=f"bm{dch}")
                nc.sync.dma_start(out=t[:], in_=b_d[dch].bitcast(F32R))
                bmat[dch] = t
            cmat = {}
            for sch in range(8):
                t = persist.tile([128, O], F32R, tag=f"cm{sch}")
                nc.sync.dma_start(out=t[:], in_=c_d[sch].bitcast(F32R))
                cmat[sch] = t
            mmat = {}
            for i in range(KX):
                for dch in range(2):
                    t = persist.tile([128, O], F32R, tag=f"mm{i}{dch}")
                    nc.sync.dma_start(out=t[:], in_=m_d[i, dch].bitcast(F32R))
                    mmat[i, dch] = t
            ah = persist.tile([128, 16], F32, tag="ah")
            nc.sync.dma_start(out=ah[:], in_=ah_d[:])

            # ---- per-batch pipeline ----
            for b in range(B_PER_CORE):
                hts = []
                for sch in range(8):
                    ht = ht_pool.tile([128, SEQ + HPAD], F32R, tag="ht")
                    nc.sync.dma_start(out=ht[:, SEQ:],
                                      in_=z_d[:].bitcast(F32R))
                    a_bc = ah[:, sch:sch + 1].broadcast_to([128, TCH])
                    for tch in range(SEQ // TCH):
                        t0 = tch * TCH
                        ub = ub_psum.tile([128, TCH], F32)
                        for dch in range(2):
                            nc.tensor.matmul(
                                out=ub[:],
                                lhsT=bmat[dch][:, sch * 128:(sch + 1) * 128],
                                rhs=xt[b, dch][:, PAD + t0:PAD + t0 + TCH],
                                start=(dch == 0),
                                stop=(dch == 1),
                            )
                        init = (ah[:, 8 + sch:9 + sch] if tch == 0
                                else ht[:, t0 - 1:t0])
                        nc.vector.tensor_tensor_scan(
                            out=ht[:, t0:t0 + TCH],
                            data0=a_bc,
                            data1=ub[:],
                            initial=init,
                            op0=mybir.AluOpType.mult,
                            op1=mybir.AluOpType.add,
                        )
                    hts.append(ht)

                for tch in range(SEQ // OTCH):
                    t0 = tch * OTCH
                    ops = out_psum.tile([128, O], F32)
                    for sch in range(8):
                        nc.tensor.matmul(
                            out=ops[:],
                            lhsT=hts[sch][:, t0 + KX:t0 + KX + OTCH],
                            rhs=cmat[sch][:],
                            start=(sch == 0),
                            stop=False,
                        )
                    for i in range(KX):
                        for dch in range(2):
                            nc.tensor.matmul(
                                out=ops[:],
                                lhsT=xt[b, dch][:, PAD - i + t0:
                                                PAD - i + t0 + OTCH],
                                rhs=mmat[i, dch][:],
                                start=False,
                                stop=(i == KX - 1 and dch == 1),
                            )
                    osb = out_sbuf.tile([128, O], F32)
                    nc.scalar.copy(out=osb[:], in_=ops[:])
                    nc.sync.dma_start(out=out_d[b, t0:t0 + OTCH, :],
                                      in_=osb[:])

    # Matmult (esp. fused-LDW f32r) supports a limited number of HW sync-wait
    # slots; split excess waits into event-semaphore chains the way
    # Bacc.compile() does.
    import bass_rust as _br
    _br.move_matmul_waits_to_ldweights(nc.m)
    _br.generate_event_semaphores(nc)

    return nc


def _prep_core_inputs(inputs, h0, A, B, C, M, core):
    """Host-side shard + layout prep for one core."""
    bs = slice(core * B_PER_CORE, (core + 1) * B_PER_CORE)
    x = inputs[bs]  # [2, T, D]
    xt = np.zeros((B_PER_CORE, 2, 128, PAD + SEQ), np.float32)
    xtr = np.ascontiguousarray(x.transpose(0, 2, 1))  # [2, D, T]
    xt[:, :, :, PAD:] = xtr.reshape(B_PER_CORE, 2, 128, SEQ)

    bmat = np.ascontiguousarray(B.reshape(2, 128, S), np.float32)
    cmat = np.ascontiguousarray(C.reshape(8, 128, O), np.float32)
    # mmat[i, dch, d, o] = M[o, dch*128+d, i]
    mmat = np.ascontiguousarray(
        M.transpose(2, 1, 0).reshape(KX, 2, 128, O), np.float32)
    ah = np.zeros((128, 16), np.float32)
    ah[:, :8] = A.reshape(8, 128).T
    ah[:, 8:] = h0.reshape(8, 128).T
    return {"xt": xt, "bmat": bmat, "cmat": cmat, "mmat": mmat, "ah": ah,
            "zt": np.zeros((128, HPAD), np.float32)}


LAST_RESULT = None


def kernel(inputs, h0, A, B, C, M):
    global LAST_RESULT
    from concourse.bass_utils import run_bass_kernel_spmd

    inputs = np.asarray(inputs, np.float32)
    h0 = np.asarray(h0, np.float32)
    A = np.asarray(A, np.float32)
    B = np.asarray(B, np.float32)
    C = np.asarray(C, np.float32)
    M = np.asarray(M, np.float32)

    if "nc" not in _CACHED:
        _CACHED["nc"] = _build_nc()
    nc = _CACHED["nc"]

    in_maps = [_prep_core_inputs(inputs, h0, A, B, C, M, c)
               for c in range(N_CORES)]
    res = run_bass_kernel_spmd(nc, in_maps, list(range(N_CORES)))
    LAST_RESULT = res
    out = np.concatenate([res.results[c]["out"] for c in range(N_CORES)],
                         axis=0)
    return out



# revision 6
# speedup vs baseline: 1.2032x; 1.2032x over previous
"""LDS (diagonal linear state space + AR) kernel for 8 Trainium2 cores.

Computation (per batch b):
    uB[t, s]   = sum_d x[t, d] * B[d, s]
    h[t]       = A * h[t-1] + uB[t]          (h[-1] = h0, A diagonal)
    lds[t, o]  = sum_s h[t, s] * C[s, o]
    out[t, o]  = sum_{i<10} sum_d M[o, d, i] * x[t-i, d]  +  lds[t+10, o]

Sharding: data-parallel over batch, 2 batches per core, no collectives.

v2 layout (all PE operands bf16; tolerance is 2e-2, bf16 error ~2e-3):
  - x host-transposed to xT [b, dch, 128d, PAD+T] bf16, DMA'd in t-slabs
  - uB matmuls use B chunks as stationary weights, stream 256-col t-chunks
    into 8 half-bank PSUM tiles (one per s-chunk); the recurrence runs as
    tensor_tensor_scan chains interleaved 8-apart so the DVE never bubbles
  - output tiles are [128o, 512t] (one PSUM bank each): stationary weights
    are M taps [128d,128o] and C chunks [128s,128o], streaming 512 t-cols;
    AR matmuls depend only on x, so they statically fill the PE gaps while
    scans drain; C matmuls follow once h is written
  - out written to HBM as [b, och, 128o, T] (contiguous rows); the host
    transposes back to [b, T, O] for free
"""

import sys

if "/opt/trn_rl_repo" not in sys.path:
    sys.path.insert(0, "/opt/trn_rl_repo")

import ml_dtypes
import numpy as np

import concourse.bass as bass
import concourse.mybir as mybir
from concourse.tile import TileContext

BSZ = 16
SEQ = 2048
D = 256  # input dim
S = 1024  # state dim
O = 256  # output dim
KX = 10
N_CORES = 8
B_PER_CORE = BSZ // N_CORES  # 2

PAD = 16  # left zero-pad on time for the AR taps (needs >= KX-1 = 9)
HPAD = 16  # right zero-pad on h time for the +10 shift (needs >= KX)
UCH = 256  # uB matmul / scan chunk width (half PSUM bank of fp32)
TCH = 512  # output tile time width (full PSUM bank of fp32)
NUC = SEQ // UCH  # 8
NTC = SEQ // TCH  # 4
FILL_PER_CHUNK = 16  # AR matmuls emitted per uB chunk to fill scan waits

F32 = mybir.dt.float32
BF16 = mybir.dt.bfloat16

_CACHED = {}


def _build_nc():
    nc = bass.Bass()

    xt_d = nc.dram_tensor("xt", [B_PER_CORE, 2, 128, PAD + SEQ], BF16,
                          kind="ExternalInput")
    bw_d = nc.dram_tensor("bw", [2, 128, 8 * 128], BF16, kind="ExternalInput")
    cw_d = nc.dram_tensor("cw", [8, 128, 2 * 128], BF16, kind="ExternalInput")
    mw_d = nc.dram_tensor("mw", [2, 128, KX * 2 * 128], BF16,
                          kind="ExternalInput")
    ah_d = nc.dram_tensor("ah", [128, 16], F32, kind="ExternalInput")
    zt_d = nc.dram_tensor("zt", [128, HPAD], BF16, kind="ExternalInput")
    out_d = nc.dram_tensor("out", [B_PER_CORE, 2, 128, SEQ], F32,
                           kind="ExternalOutput")

    # xT DMA slabs: first uB/AR chunk only needs cols [0, PAD+UCH)
    slabs = [(0, PAD + UCH)]
    c = PAD + UCH
    while c < PAD + SEQ:
        e = min(c + 512, PAD + SEQ)
        slabs.append((c, e))
        c = e

    with TileContext(nc) as tc:
        with tc.tile_pool(name="persist", bufs=1) as persist, \
             tc.tile_pool(name="ht", bufs=16) as ht_pool, \
             tc.tile_pool(name="outsb", bufs=6) as out_sbuf, \
             tc.tile_pool(name="ubps", bufs=4, space="PSUM") as ub_psum, \
             tc.tile_pool(name="outps", bufs=4, space="PSUM") as out_psum:

            # ---- persistent operands; emission order = DMA priority ----
            bwt = {}
            for dch in range(2):
                t = persist.tile([128, 8 * 128], BF16, tag=f"bw{dch}")
                nc.sync.dma_start(out=t[:], in_=bw_d[dch])
                bwt[dch] = t
            xtt = {}
            for b in range(B_PER_CORE):
                for dch in range(2):
                    xtt[b, dch] = persist.tile([128, PAD + SEQ], BF16,
                                               name=f"xt{b}{dch}",
                                               tag=f"xt{b}{dch}")
            for dch in range(2):  # batch-0 first slab: unblocks chunk 0
                s0, s1 = slabs[0]
                nc.sync.dma_start(out=xtt[0, dch][:, s0:s1],
                                  in_=xt_d[0, dch][:, s0:s1])
            ah = persist.tile([128, 16], F32, tag="ah")
            nc.sync.dma_start(out=ah[:], in_=ah_d[:])
            mwt = {}
            for dch in range(2):
                t = persist.tile([128, KX * 2 * 128], BF16, tag=f"mw{dch}")
                nc.sync.dma_start(out=t[:], in_=mw_d[dch])
                mwt[dch] = t
            for s0, s1 in slabs[1:]:
                for dch in range(2):
                    nc.sync.dma_start(out=xtt[0, dch][:, s0:s1],
                                      in_=xt_d[0, dch][:, s0:s1])
            cwt = {}
            for sch in range(8):
                t = persist.tile([128, 2 * 128], BF16, tag=f"cw{sch}")
                nc.sync.dma_start(out=t[:], in_=cw_d[sch])
                cwt[sch] = t
            for s0, s1 in slabs:
                for dch in range(2):
                    nc.sync.dma_start(out=xtt[1, dch][:, s0:s1],
                                      in_=xt_d[1, dch][:, s0:s1])

            hts = {}

            # ---- output-side work units (the PE fill queue) ----
            OP = {}

            def ar_unit(b, tch, och, w):
                dch, i = divmod(w, KX)
                t0 = tch * TCH
                key = (b, tch, och)
                first = key not in OP
                if first:
                    OP[key] = out_psum.tile([128, TCH], F32, name="op",
                                            tag="op")
                j = i * 2 + och
                nc.tensor.matmul(
                    out=OP[key][:],
                    lhsT=mwt[dch][:, j * 128:(j + 1) * 128],
                    rhs=xtt[b, dch][:, PAD + t0 - i:PAD + t0 - i + TCH],
                    start=first, stop=False,
                )

            def c_unit(b, tch, och, sch):
                t0 = tch * TCH
                nc.tensor.matmul(
                    out=OP[(b, tch, och)][:],
                    lhsT=cwt[sch][:, och * 128:(och + 1) * 128],
                    rhs=hts[b, sch][:, t0 + KX:t0 + KX + TCH],
                    start=False, stop=(sch == 7),
                )

            def out_unit(b, tch, och):
                osb = out_sbuf.tile([128, TCH], F32)
                nc.scalar.copy(out=osb[:], in_=OP[(b, tch, och)][:])
                nc.sync.dma_start(out=out_d[b, och][:, tch * TCH:
                                                     (tch + 1) * TCH],
                                  in_=osb[:])
                del OP[(b, tch, och)]

            def batch_fifo(b):
                # (gate, fn): gate=b means "after all scans of batch b
                # are emitted" (C reads h; emission order defines deps)
                q = []
                for och in range(2):
                    for w in range(2 * KX):
                        q.append((None, lambda b=b, o=och, w=w:
                                  ar_unit(b, 0, o, w)))
                for och in range(2):
                    for w in range(2 * KX):
                        q.append((None, lambda b=b, o=och, w=w:
                                  ar_unit(b, 1, o, w)))
                for tch in range(NTC):
                    if tch >= 2:
                        for och in range(2):
                            for w in range(2 * KX):
                                q.append((None, lambda b=b, t=tch, o=och,
                                          w=w: ar_unit(b, t, o, w)))
                    for och in range(2):
                        for sch in range(8):
                            q.append((b, lambda b=b, t=tch, o=och, s=sch:
                                      c_unit(b, t, o, s)))
                    for och in range(2):
                        q.append((b, lambda b=b, t=tch, o=och:
                                  out_unit(b, t, o)))
                return q

            fifo = batch_fifo(0) + batch_fifo(1)
            gates_open = set()
            cursor = [0]

            def fill(n):
                k = 0
                while k < n and cursor[0] < len(fifo):
                    g, fn = fifo[cursor[0]]
                    if g is not None and g not in gates_open:
                        break
                    fn()
                    cursor[0] += 1
                    k += 1

            # ---- uB + scan pipeline, AR matmuls filling the PE gaps ----
            for b in range(B_PER_CORE):
                for sch in range(8):
                    ht = ht_pool.tile([128, SEQ + HPAD], BF16, tag="ht")
                    nc.sync.dma_start(out=ht[:, SEQ:], in_=zt_d[:])
                    hts[b, sch] = ht
                for ch in range(NUC):
                    t0 = ch * UCH
                    for j in range(4):  # PSUM bank holds 2 s-chunks
                        ub = ub_psum.tile([128, 2 * UCH], F32)
                        for k in range(2):
                            sch = 2 * j + k
                            ubs = ub[:, k * UCH:(k + 1) * UCH]
                            for dch in range(2):
                                nc.tensor.matmul(
                                    out=ubs,
                                    lhsT=bwt[dch][:, sch * 128:
                                                  (sch + 1) * 128],
                                    rhs=xtt[b, dch][:, PAD + t0:
                                                    PAD + t0 + UCH],
                                    start=(dch == 0), stop=(dch == 1),
                                )
                            init = (ah[:, 8 + sch:9 + sch] if ch == 0
                                    else hts[b, sch][:, t0 - 1:t0])
                            nc.vector.tensor_tensor_scan(
                                out=hts[b, sch][:, t0:t0 + UCH],
                                data0=ah[:, sch:sch + 1]
                                .broadcast_to([128, UCH]),
                                data1=ubs,
                                initial=init,
                                op0=mybir.AluOpType.mult,
                                op1=mybir.AluOpType.add,
                            )
                    fill(FILL_PER_CHUNK)
                gates_open.add(b)
            fill(len(fifo))

    # Matmult (esp. fused-LDW) supports a limited number of HW sync-wait
    # slots; split excess waits into event-semaphore chains the way
    # Bacc.compile() does.
    import bass_rust as _br
    _br.move_matmul_waits_to_ldweights(nc.m)
    _br.generate_event_semaphores(nc)

    return nc


def _prep_core_inputs(inputs, h0, A, B, C, M, core):
    """Host-side shard + layout prep for one core."""
    bf16 = ml_dtypes.bfloat16
    bs = slice(core * B_PER_CORE, (core + 1) * B_PER_CORE)
    x = inputs[bs]  # [2, T, D]
    xt = np.zeros((B_PER_CORE, 2, 128, PAD + SEQ), bf16)
    xtr = np.ascontiguousarray(x.transpose(0, 2, 1))  # [2, D, T]
    xt[:, :, :, PAD:] = xtr.reshape(B_PER_CORE, 2, 128, SEQ).astype(bf16)

    bw = np.ascontiguousarray(B.reshape(2, 128, 8 * 128)).astype(bf16)
    cw = np.ascontiguousarray(C.reshape(8, 128, 2 * 128)).astype(bf16)
    # mw[dch, d, (i*2+och)*128+o] = M[och*128+o, dch*128+d, i]
    mw = np.ascontiguousarray(
        M.transpose(1, 2, 0).reshape(2, 128, KX, 2, 128)
        .transpose(0, 1, 2, 3, 4).reshape(2, 128, KX * 2 * 128)).astype(bf16)
    ah = np.zeros((128, 16), np.float32)
    ah[:, :8] = A.reshape(8, 128).T
    ah[:, 8:] = h0.reshape(8, 128).T
    return {"xt": xt, "bw": bw, "cw": cw, "mw": mw, "ah": ah,
            "zt": np.zeros((128, HPAD), bf16)}


LAST_RESULT = None


def kernel(inputs, h0, A, B, C, M):
    global LAST_RESULT
    from concourse.bass_utils import run_bass_kernel_spmd

    inputs = np.asarray(inputs, np.float32)
    h0 = np.asarray(h0, np.float32)
    A = np.asarray(A, np.float32)
    B = np.asarray(B, np.float32)
    C = np.asarray(C, np.float32)
    M = np.asarray(M, np.float32)

    if "nc" not in _CACHED:
        _CACHED["nc"] = _build_nc()
    nc = _CACHED["nc"]

    in_maps = [_prep_core_inputs(inputs, h0, A, B, C, M, c)
               for c in range(N_CORES)]
    res = run_bass_kernel_spmd(nc, in_maps, list(range(N_CORES)))
    LAST_RESULT = res
    # res: [b, och, 128o, T] per core -> [b, T, O]
    out = np.concatenate(
        [res.results[c]["out"].transpose(0, 3, 1, 2).reshape(
            B_PER_CORE, SEQ, O) for c in range(N_CORES)], axis=0)
    return np.ascontiguousarray(out, np.float32)


# revision 9
# speedup vs baseline: 1.3495x; 1.1216x over previous
"""LDS (diagonal linear state space + AR) kernel for 8 Trainium2 cores.

Computation (per batch b):
    uB[t, s]   = sum_d x[t, d] * B[d, s]
    h[t]       = A * h[t-1] + uB[t]          (h[-1] = h0, A diagonal)
    lds[t, o]  = sum_s h[t, s] * C[s, o]
    out[t, o]  = sum_{i<10} sum_d M[o, d, i] * x[t-i, d]  +  lds[t+10, o]

Sharding: data-parallel over batch, 2 batches per core, no collectives.

v3 layout (all PE operands bf16; tolerance is 2e-2, bf16 error ~2.5e-3):
  - x host-transposed to xT [b, dch, 128d, PAD+T] bf16, DMA'd in t-slabs
  - uB matmuls use B chunks as stationary weights streaming 512-col
    chunk-pairs into full-bank PSUM tiles (one per s-chunk, ring of 4);
    the recurrence runs as 256-col tensor_tensor_scan chains interleaved
    4-apart so the DVE pipeline never bubbles
  - output tiles are [128o, 512t] (one PSUM bank each): stationary weights
    are M taps [128d,128o] and C chunks [128s,128o], each loaded once per
    time-tile PAIR (2x 512-col streams per LDW); AR matmuls depend only on
    x, so they statically fill the PE gaps while scans drain; C matmuls
    are gated per scan-chunk and follow as soon as h is written
  - out written to HBM as [b, och, 128o, T] (contiguous rows); the host
    transposes back to [b, T, O] for free
"""

import sys

if "/opt/trn_rl_repo" not in sys.path:
    sys.path.insert(0, "/opt/trn_rl_repo")

import ml_dtypes
import numpy as np

import concourse.bass as bass
import concourse.mybir as mybir
from concourse.tile import TileContext

BSZ = 16
SEQ = 2048
D = 256  # input dim
S = 1024  # state dim
O = 256  # output dim
KX = 10
N_CORES = 8
B_PER_CORE = BSZ // N_CORES  # 2

PAD = 16  # left zero-pad on time for the AR taps (needs >= KX-1 = 9)
HPAD = 16  # right zero-pad on h time for the +10 shift (needs >= KX)
UCH = 256  # scan chunk width
TCH = 512  # matmul stream width / output tile time width (= 1 PSUM bank)
NCP = SEQ // TCH  # 4 uB chunk-pairs per batch
NTC = SEQ // TCH  # 4 output time tiles per batch
FILL_PER_HALF = 9  # fill units emitted per uB half-round (4 s-chunks)

F32 = mybir.dt.float32
BF16 = mybir.dt.bfloat16

_CACHED = {}


def _build_nc():
    nc = bass.Bass()

    xt_d = nc.dram_tensor("xt", [B_PER_CORE, 2, 128, PAD + SEQ], BF16,
                          kind="ExternalInput")
    bw_d = nc.dram_tensor("bw", [2, 128, 8 * 128], BF16, kind="ExternalInput")
    cw_d = nc.dram_tensor("cw", [8, 128, 2 * 128], BF16, kind="ExternalInput")
    mw_d = nc.dram_tensor("mw", [2, 128, KX * 2 * 128], BF16,
                          kind="ExternalInput")
    ah_d = nc.dram_tensor("ah", [128, 16], F32, kind="ExternalInput")
    zt_d = nc.dram_tensor("zt", [128, HPAD], BF16, kind="ExternalInput")
    out_d = nc.dram_tensor("out", [B_PER_CORE, 2, 128, SEQ], F32,
                           kind="ExternalOutput")

    # xT DMA slabs: slab k covers chunk-pair k's reads (and AR windows)
    slabs = [(0, PAD + TCH)]
    c = PAD + TCH
    while c < PAD + SEQ:
        e = min(c + TCH, PAD + SEQ)
        slabs.append((c, e))
        c = e

    with TileContext(nc) as tc:
        with tc.tile_pool(name="persist", bufs=1) as persist, \
             tc.tile_pool(name="ht", bufs=16) as ht_pool, \
             tc.tile_pool(name="outsb", bufs=6) as out_sbuf, \
             tc.tile_pool(name="ubps", bufs=4, space="PSUM") as ub_psum, \
             tc.tile_pool(name="outps", bufs=4, space="PSUM") as out_psum:

            # ---- persistent operands; emission order = DMA priority ----
            bwt = {}
            for dch in range(2):
                t = persist.tile([128, 8 * 128], BF16, tag=f"bw{dch}")
                nc.sync.dma_start(out=t[:], in_=bw_d[dch])
                bwt[dch] = t
            xtt = {}
            for b in range(B_PER_CORE):
                for dch in range(2):
                    xtt[b, dch] = persist.tile([128, PAD + SEQ], BF16,
                                               name=f"xt{b}{dch}",
                                               tag=f"xt{b}{dch}")
            for dch in range(2):  # batch-0 first slab: unblocks round 0
                s0, s1 = slabs[0]
                nc.sync.dma_start(out=xtt[0, dch][:, s0:s1],
                                  in_=xt_d[0, dch][:, s0:s1])
            ah = persist.tile([128, 16], F32, tag="ah")
            nc.sync.dma_start(out=ah[:], in_=ah_d[:])
            mwt = {}
            for dch in range(2):  # AR fills start in round 0
                t = persist.tile([128, KX * 2 * 128], BF16, tag=f"mw{dch}")
                nc.sync.dma_start(out=t[:], in_=mw_d[dch])
                mwt[dch] = t
            for s0, s1 in slabs[1:]:
                for dch in range(2):
                    nc.sync.dma_start(out=xtt[0, dch][:, s0:s1],
                                      in_=xt_d[0, dch][:, s0:s1])
            cwt = {}
            for sch in range(8):
                t = persist.tile([128, 2 * 128], BF16, tag=f"cw{sch}")
                nc.sync.dma_start(out=t[:], in_=cw_d[sch])
                cwt[sch] = t
            for s0, s1 in slabs:
                for dch in range(2):
                    nc.sync.dma_start(out=xtt[1, dch][:, s0:s1],
                                      in_=xt_d[1, dch][:, s0:s1])

            hts = {}

            # ---- output-side work units (the PE fill queue) ----
            # one unit = one stationary weight streaming both tiles of a
            # time-tile pair (tp: tiles 2tp and 2tp+1)
            OP = {}

            def op_tile(b, tch, och):
                key = (b, tch, och)
                first = key not in OP
                if first:
                    OP[key] = out_psum.tile([128, TCH], F32, name="op",
                                            tag="op")
                return OP[key], first

            def ar_unit(b, tp, och, w):
                dch, i = divmod(w, KX)
                j = i * 2 + och
                lhs = mwt[dch][:, j * 128:(j + 1) * 128]
                for tch in (2 * tp, 2 * tp + 1):
                    t0 = tch * TCH
                    op, first = op_tile(b, tch, och)
                    nc.tensor.matmul(
                        out=op[:], lhsT=lhs,
                        rhs=xtt[b, dch][:, PAD + t0 - i:PAD + t0 - i + TCH],
                        start=first, stop=False,
                    )

            def c_unit(b, tch, och, sch):
                t0 = tch * TCH
                op, _ = op_tile(b, tch, och)
                nc.tensor.matmul(
                    out=op[:],
                    lhsT=cwt[sch][:, och * 128:(och + 1) * 128],
                    rhs=hts[b, sch][:, t0 + KX:t0 + KX + TCH],
                    start=False, stop=(sch == 7),
                )

            def out_unit(b, tch, och):
                osb = out_sbuf.tile([128, TCH], F32)
                nc.scalar.copy(out=osb[:], in_=OP[(b, tch, och)][:])
                nc.sync.dma_start(out=out_d[b, och][:, tch * TCH:
                                                     (tch + 1) * TCH],
                                  in_=osb[:])
                del OP[(b, tch, och)]

            def batch_fifo(b):
                # (gate, fn): gate=(b, creq) means "after scan chunk creq
                # of every s-chunk of batch b is emitted" (C reads h;
                # emission order defines deps). tile tch reads h up to
                # t=tch*TCH + KX + TCH - 1 -> chunk creq; reads beyond
                # chunk 7 land in the DMA'd zero tail.
                q = []
                for tp in range(2):
                    for och in range(2):
                        for w in range(2 * KX):
                            q.append((None, lambda b=b, tp=tp, o=och, w=w:
                                      ar_unit(b, tp, o, w)))
                    for tch in (2 * tp, 2 * tp + 1):
                        creq = min((tch * TCH + KX + TCH - 1) // UCH,
                                   SEQ // UCH - 1)
                        for och in range(2):
                            for sch in range(8):
                                q.append(((b, creq),
                                          lambda b=b, t=tch, o=och, s=sch:
                                          c_unit(b, t, o, s)))
                        for och in range(2):
                            q.append(((b, creq), lambda b=b, t=tch, o=och:
                                      out_unit(b, t, o)))
                return q

            fifo = batch_fifo(0) + batch_fifo(1)
            chunks_done = {0: -1, 1: -1}
            cursor = [0]

            def fill(n):
                k = 0
                while k < n and cursor[0] < len(fifo):
                    g, fn = fifo[cursor[0]]
                    if g is not None and chunks_done[g[0]] < g[1]:
                        break
                    fn()
                    cursor[0] += 1
                    k += 1

            # ---- uB + scan pipeline, AR/C matmuls filling the PE gaps ----
            for b in range(B_PER_CORE):
                for sch in range(8):
                    ht = ht_pool.tile([128, SEQ + HPAD], BF16, tag="ht")
                    nc.sync.dma_start(out=ht[:, SEQ:], in_=zt_d[:])
                    hts[b, sch] = ht
                for cp in range(NCP):
                    t0 = cp * TCH
                    for half in range(2):
                        ubt = {}
                        for sj in range(4):
                            sch = half * 4 + sj
                            ub = ub_psum.tile([128, TCH], F32)
                            for dch in range(2):
                                nc.tensor.matmul(
                                    out=ub[:],
                                    lhsT=bwt[dch][:, sch * 128:
                                                  (sch + 1) * 128],
                                    rhs=xtt[b, dch][:, PAD + t0:
                                                    PAD + t0 + TCH],
                                    start=(dch == 0), stop=(dch == 1),
                                )
                            ubt[sch] = ub
                        # scans: chains interleaved 4-apart, no bubbles
                        for half_ch in range(2):
                            tc0 = t0 + half_ch * UCH
                            for sj in range(4):
                                sch = half * 4 + sj
                                init = (ah[:, 8 + sch:9 + sch]
                                        if cp == 0 and half_ch == 0
                                        else hts[b, sch][:, tc0 - 1:tc0])
                                nc.vector.tensor_tensor_scan(
                                    out=hts[b, sch][:, tc0:tc0 + UCH],
                                    data0=ah[:, sch:sch + 1]
                                    .broadcast_to([128, UCH]),
                                    data1=ubt[sch][:, half_ch * UCH:
                                                   (half_ch + 1) * UCH],
                                    initial=init,
                                    op0=mybir.AluOpType.mult,
                                    op1=mybir.AluOpType.add,
                                )
                        fill(FILL_PER_HALF)
                    chunks_done[b] = 2 * cp + 1
            fill(len(fifo))

    # Matmult (esp. fused-LDW) supports a limited number of HW sync-wait
    # slots; split excess waits into event-semaphore chains the way
    # Bacc.compile() does.
    import bass_rust as _br
    _br.move_matmul_waits_to_ldweights(nc.m)
    _br.generate_event_semaphores(nc)

    return nc


def _prep_core_inputs(inputs, h0, A, B, C, M, core):
    """Host-side shard + layout prep for one core."""
    bf16 = ml_dtypes.bfloat16
    bs = slice(core * B_PER_CORE, (core + 1) * B_PER_CORE)
    x = inputs[bs]  # [2, T, D]
    xt = np.zeros((B_PER_CORE, 2, 128, PAD + SEQ), bf16)
    xtr = np.ascontiguousarray(x.transpose(0, 2, 1))  # [2, D, T]
    xt[:, :, :, PAD:] = xtr.reshape(B_PER_CORE, 2, 128, SEQ).astype(bf16)

    bw = np.ascontiguousarray(B.reshape(2, 128, 8 * 128)).astype(bf16)
    cw = np.ascontiguousarray(C.reshape(8, 128, 2 * 128)).astype(bf16)
    # mw[dch, d, (i*2+och)*128+o] = M[och*128+o, dch*128+d, i]
    mw = np.ascontiguousarray(
        M.transpose(1, 2, 0).reshape(2, 128, KX, 2, 128)
        .reshape(2, 128, KX * 2 * 128)).astype(bf16)
    ah = np.zeros((128, 16), np.float32)
    ah[:, :8] = A.reshape(8, 128).T
    ah[:, 8:] = h0.reshape(8, 128).T
    return {"xt": xt, "bw": bw, "cw": cw, "mw": mw, "ah": ah,
            "zt": np.zeros((128, HPAD), bf16)}


LAST_RESULT = None


def kernel(inputs, h0, A, B, C, M):
    global LAST_RESULT
    from concourse.bass_utils import run_bass_kernel_spmd

    inputs = np.asarray(inputs, np.float32)
    h0 = np.asarray(h0, np.float32)
    A = np.asarray(A, np.float32)
    B = np.asarray(B, np.float32)
    C = np.asarray(C, np.float32)
    M = np.asarray(M, np.float32)

    if "nc" not in _CACHED:
        _CACHED["nc"] = _build_nc()
    nc = _CACHED["nc"]

    in_maps = [_prep_core_inputs(inputs, h0, A, B, C, M, c)
               for c in range(N_CORES)]
    res = run_bass_kernel_spmd(nc, in_maps, list(range(N_CORES)))
    LAST_RESULT = res
    # res: [b, och, 128o, T] per core -> [b, T, O]
    out = np.concatenate(
        [res.results[c]["out"].transpose(0, 3, 1, 2).reshape(
            B_PER_CORE, SEQ, O) for c in range(N_CORES)], axis=0)
    return np.ascontiguousarray(out, np.float32)


# revision 10
# speedup vs baseline: 1.5683x; 1.1622x over previous
"""LDS (diagonal linear state space + AR) kernel for 8 Trainium2 cores.

Computation (per batch b):
    uB[t, s]   = sum_d x[t, d] * B[d, s]
    h[t]       = A * h[t-1] + uB[t]          (h[-1] = h0, A diagonal)
    lds[t, o]  = sum_s h[t, s] * C[s, o]
    out[t, o]  = sum_{i<10} sum_d M[o, d, i] * x[t-i, d]  +  lds[t+10, o]

Sharding: data-parallel over batch, 2 batches per core, no collectives.

v4: the AR term dominates the output (~100x the lds term), so the AR
matmuls run in bf16 while the entire lds branch (uB, C) runs in fp8
DoubleRow mode (K=256 per matmul, 2x PE throughput). Scales are exact
powers of two folded through the linear recurrence: B*32 and x in fp8
give uB' = 32*uB; the f32 scan carries h' = 32*h and writes fp8 h8;
C*256 in fp8 gives lds' = 8192*lds; the AR weights M*8192 in bf16 put
both terms at the same scale in one PSUM tile, descaled by the final
scaled-copy. End-to-end error stays ~2.5e-3 (vs 2e-2 tolerance).

  - x lives twice in SBUF: bf16 [dch, 128d, PAD+T] for AR, and fp8
    DR-packed [128, 2, PAD+T] for uB; both DMA'd in t-slabs
  - uB: one DoubleRow matmul per (s-chunk, 512-col chunk) into a PSUM
    bank; 512-col f32 scans chain 8-apart (no DVE bubbles), writing
    fp8 h8 tiles [128, 2, T] packed for the C DoubleRow matmuls
  - output tiles [128o, 512t]: 20 bf16 AR matmuls (stationary M taps)
    + 4 fp8-DR C matmuls accumulate; AR matmuls depend only on x and
    statically fill the PE gaps while scans drain; C matmuls are gated
    per scan-chunk
  - out written to HBM as [b, och, 128o, T]; host transposes for free
"""

import sys

if "/opt/trn_rl_repo" not in sys.path:
    sys.path.insert(0, "/opt/trn_rl_repo")

import ml_dtypes
import numpy as np

import concourse.bass as bass
import concourse.mybir as mybir
from concourse.tile import TileContext

BSZ = 16
SEQ = 2048
D = 256  # input dim
S = 1024  # state dim
O = 256  # output dim
KX = 10
N_CORES = 8
B_PER_CORE = BSZ // N_CORES  # 2

PAD = 16  # left zero-pad on time for the AR taps (needs >= KX-1 = 9)
HPAD = 16  # right zero-pad on h time for the +10 shift (needs >= KX)
TCH = 512  # matmul stream width / chunk width (= 1 PSUM bank of f32)
NCP = SEQ // TCH  # 4 chunks per batch
FILL_PER_HALF = 10  # fill units emitted per uB half-round (4 s-chunks)

SC_B = 32.0  # B scale -> h' = 32*h (|h'| < ~150, fp8 max 240)
SC_C = 256.0  # C scale -> lds' = 8192*lds
SC_OUT = SC_B * SC_C  # = 8192; M scaled by this in bf16 (exact)

F32 = mybir.dt.float32
BF16 = mybir.dt.bfloat16
F8 = mybir.dt.float8e4
DR = mybir.MatmulPerfMode.DoubleRow

_CACHED = {}


def _build_nc():
    nc = bass.Bass()

    xt_d = nc.dram_tensor("xt", [B_PER_CORE, 2, 128, PAD + SEQ], BF16,
                          kind="ExternalInput")
    x8_d = nc.dram_tensor("x8", [B_PER_CORE, 128, 2, PAD + SEQ], F8,
                          kind="ExternalInput")
    bw_d = nc.dram_tensor("bw", [128, 8, 2, 128], F8, kind="ExternalInput")
    cw_d = nc.dram_tensor("cw", [128, 4, 2, 2, 128], F8,
                          kind="ExternalInput")
    mw_d = nc.dram_tensor("mw", [2, 128, KX * 2 * 128], BF16,
                          kind="ExternalInput")
    ah_d = nc.dram_tensor("ah", [128, 16], F32, kind="ExternalInput")
    z8_d = nc.dram_tensor("z8", [128, 2, HPAD], F8, kind="ExternalInput")
    out_d = nc.dram_tensor("out", [B_PER_CORE, 2, 128, SEQ], F32,
                           kind="ExternalOutput")

    # x DMA slabs: slab k covers chunk k's reads (and AR windows)
    slabs = [(0, PAD + TCH)]
    c = PAD + TCH
    while c < PAD + SEQ:
        e = min(c + TCH, PAD + SEQ)
        slabs.append((c, e))
        c = e

    with TileContext(nc) as tc:
        with tc.tile_pool(name="persist", bufs=1) as persist, \
             tc.tile_pool(name="h8p", bufs=8) as h8_pool, \
             tc.tile_pool(name="outsb", bufs=6) as out_sbuf, \
             tc.tile_pool(name="ubps", bufs=4, space="PSUM") as ub_psum, \
             tc.tile_pool(name="outps", bufs=4, space="PSUM") as out_psum:

            # ---- persistent operands; emission order = DMA priority ----
            bw8 = persist.tile([128, 8, 2, 128], F8, tag="bw8")
            nc.sync.dma_start(out=bw8[:], in_=bw_d[:])
            x8t = {}
            for b in range(B_PER_CORE):
                x8t[b] = persist.tile([128, 2, PAD + SEQ], F8,
                                      name=f"x8{b}", tag=f"x8{b}")
            xtt = {}
            for b in range(B_PER_CORE):
                for dch in range(2):
                    xtt[b, dch] = persist.tile([128, PAD + SEQ], BF16,
                                               name=f"xt{b}{dch}",
                                               tag=f"xt{b}{dch}")
            s0, s1 = slabs[0]
            nc.sync.dma_start(out=x8t[0][:, :, s0:s1],
                              in_=x8_d[0][:, :, s0:s1])
            ah = persist.tile([128, 16], F32, tag="ah")
            nc.sync.dma_start(out=ah[:], in_=ah_d[:])
            mwt = {}
            for dch in range(2):  # AR fills start in round 0
                t = persist.tile([128, KX * 2 * 128], BF16, tag=f"mw{dch}")
                nc.sync.dma_start(out=t[:], in_=mw_d[dch])
                mwt[dch] = t
            for s0, s1 in slabs[:2]:  # AR tile-pair 0 reads up to col 1040
                for dch in range(2):
                    nc.sync.dma_start(out=xtt[0, dch][:, s0:s1],
                                      in_=xt_d[0, dch][:, s0:s1])
            for s0, s1 in slabs[1:]:
                nc.sync.dma_start(out=x8t[0][:, :, s0:s1],
                                  in_=x8_d[0][:, :, s0:s1])
            cw8 = persist.tile([128, 4, 2, 2, 128], F8, tag="cw8")
            nc.sync.dma_start(out=cw8[:], in_=cw_d[:])
            for s0, s1 in slabs[2:]:
                for dch in range(2):
                    nc.sync.dma_start(out=xtt[0, dch][:, s0:s1],
                                      in_=xt_d[0, dch][:, s0:s1])
            for s0, s1 in slabs:
                nc.sync.dma_start(out=x8t[1][:, :, s0:s1],
                                  in_=x8_d[1][:, :, s0:s1])
                for dch in range(2):
                    nc.sync.dma_start(out=xtt[1, dch][:, s0:s1],
                                      in_=xt_d[1, dch][:, s0:s1])

            h8 = {}

            # ---- output-side work units (the PE fill queue) ----
            OP = {}

            def op_tile(b, tch, och):
                key = (b, tch, och)
                first = key not in OP
                if first:
                    OP[key] = out_psum.tile([128, TCH], F32, name="op",
                                            tag="op")
                return OP[key], first

            def ar_unit(b, tchs, och, w):
                # one stationary M tap streaming 1-2 time tiles
                dch, i = divmod(w, KX)
                j = i * 2 + och
                lhs = mwt[dch][:, j * 128:(j + 1) * 128]
                for tch in tchs:
                    t0 = tch * TCH
                    op, first = op_tile(b, tch, och)
                    nc.tensor.matmul(
                        out=op[:], lhsT=lhs,
                        rhs=xtt[b, dch][:, PAD + t0 - i:PAD + t0 - i + TCH],
                        start=first, stop=False,
                    )

            def c_unit(b, tch, och):
                # lds' += h8 @ C8 over 4 DoubleRow k-groups
                t0 = tch * TCH
                op, _ = op_tile(b, tch, och)
                for q in range(4):
                    nc.tensor.matmul(
                        out=op[:],
                        lhsT=cw8[:, q, :, och],
                        rhs=h8[b, q][:, :, t0 + KX:t0 + KX + TCH],
                        start=False, stop=(q == 3),
                        perf_mode=DR,
                    )

            def out_unit(b, tch, och):
                osb = out_sbuf.tile([128, TCH], F32)
                nc.scalar.activation(out=osb[:], in_=OP[(b, tch, och)][:],
                                     func=mybir.ActivationFunctionType.Copy,
                                     scale=1.0 / SC_OUT)
                nc.sync.dma_start(out=out_d[b, och][:, tch * TCH:
                                                     (tch + 1) * TCH],
                                  in_=osb[:])
                del OP[(b, tch, och)]

            def batch_fifo(b):
                # (gate, fn): gate=(b, creq) means "after scan chunk creq
                # of batch b is emitted" (C reads h; emission order defines
                # deps). AR for tiles 2/3 runs single-tile so it can slot
                # in as soon as the t0/t1 PSUM tiles retire.
                q = []

                def creq(tch):
                    return (b, min((tch * TCH + KX + TCH - 1) // TCH,
                                   NCP - 1))

                for och in range(2):
                    for w in range(2 * KX):
                        q.append((None, lambda b=b, o=och, w=w:
                                  ar_unit(b, (0, 1), o, w)))
                for och in range(2):
                    q.append((creq(0), lambda b=b, o=och: c_unit(b, 0, o)))
                for och in range(2):
                    q.append((creq(0), lambda b=b, o=och: out_unit(b, 0, o)))
                for och in range(2):
                    for w in range(2 * KX):
                        q.append((None, lambda b=b, o=och, w=w:
                                  ar_unit(b, (2,), o, w)))
                for och in range(2):
                    q.append((creq(1), lambda b=b, o=och: c_unit(b, 1, o)))
                for och in range(2):
                    q.append((creq(1), lambda b=b, o=och: out_unit(b, 1, o)))
                for och in range(2):
                    for w in range(2 * KX):
                        q.append((None, lambda b=b, o=och, w=w:
                                  ar_unit(b, (3,), o, w)))
                for tch in (2, 3):
                    for och in range(2):
                        q.append((creq(tch), lambda b=b, t=tch, o=och:
                                  c_unit(b, t, o)))
                    for och in range(2):
                        q.append((creq(tch), lambda b=b, t=tch, o=och:
                                  out_unit(b, t, o)))
                return q

            fifo = batch_fifo(0) + batch_fifo(1)
            chunks_done = {0: -1, 1: -1}
            cursor = [0]

            def fill(n):
                k = 0
                while k < n and cursor[0] < len(fifo):
                    g, fn = fifo[cursor[0]]
                    if g is not None and chunks_done[g[0]] < g[1]:
                        break
                    fn()
                    cursor[0] += 1
                    k += 1

            # ---- uB + scan pipeline, AR/C matmuls filling the PE gaps ----
            for b in range(B_PER_CORE):
                for qq in range(4):
                    t = h8_pool.tile([128, 2, SEQ + HPAD], F8, name="h8",
                                     tag="h8")
                    nc.sync.dma_start(out=t[:, :, SEQ:], in_=z8_d[:])
                    h8[b, qq] = t
                for cp in range(NCP):
                    t0 = cp * TCH
                    for half in range(2):
                        ubs = {}
                        for sj in range(4):
                            sch = half * 4 + sj
                            ub = ub_psum.tile([128, TCH], F32)
                            nc.tensor.matmul(
                                out=ub[:],
                                lhsT=bw8[:, sch],
                                rhs=x8t[b][:, :, PAD + t0:PAD + t0 + TCH],
                                start=True, stop=True,
                                perf_mode=DR,
                            )
                            ubs[sch] = ub
                        for sj in range(4):
                            sch = half * 4 + sj
                            qq, jj = divmod(sch, 2)
                            init = (ah[:, 8 + sch:9 + sch] if cp == 0
                                    else h8[b, qq][:, jj, t0 - 1:t0])
                            nc.vector.tensor_tensor_scan(
                                out=h8[b, qq][:, jj, t0:t0 + TCH],
                                data0=ah[:, sch:sch + 1]
                                .broadcast_to([128, TCH]),
                                data1=ubs[sch][:],
                                initial=init,
                                op0=mybir.AluOpType.mult,
                                op1=mybir.AluOpType.add,
                            )
                        fill(FILL_PER_HALF)
                    chunks_done[b] = cp
            fill(len(fifo))

    # Matmult (esp. fused-LDW) supports a limited number of HW sync-wait
    # slots; split excess waits into event-semaphore chains the way
    # Bacc.compile() does.
    import bass_rust as _br
    _br.move_matmul_waits_to_ldweights(nc.m)
    _br.generate_event_semaphores(nc)

    return nc


def _prep_core_inputs(inputs, h0, A, B, C, M, core):
    """Host-side shard + layout prep for one core."""
    bf16 = ml_dtypes.bfloat16
    f8 = mybir.dt.np(mybir.dt.float8e4)
    bs = slice(core * B_PER_CORE, (core + 1) * B_PER_CORE)
    x = inputs[bs]  # [2, T, D]
    xtr = np.ascontiguousarray(x.transpose(0, 2, 1))  # [2, D, T]
    xt = np.zeros((B_PER_CORE, 2, 128, PAD + SEQ), bf16)
    xt[:, :, :, PAD:] = xtr.reshape(B_PER_CORE, 2, 128, SEQ).astype(bf16)
    # x8[b, p, j, t] = x[b, t, j*128+p]
    x8 = np.zeros((B_PER_CORE, 128, 2, PAD + SEQ), f8)
    x8[:, :, :, PAD:] = xtr.reshape(B_PER_CORE, 2, 128, SEQ).transpose(
        0, 2, 1, 3).astype(f8)

    # bw[p, sch, j, s] = 32 * B[j*128+p, sch*128+s]
    bw = np.ascontiguousarray(
        (B * SC_B).reshape(2, 128, 8, 128).transpose(1, 2, 0, 3)).astype(f8)
    # cw[p, q, j, och, o] = 256 * C[(2q+j)*128+p, och*128+o]
    cw = np.ascontiguousarray(
        (C * SC_C).reshape(4, 2, 128, 2, 128).transpose(2, 0, 1, 3, 4)
    ).astype(f8)
    # mw[dch, d, (i*2+och)*128+o] = 8192 * M[och*128+o, dch*128+d, i]
    mw = np.ascontiguousarray(
        (M * SC_OUT).transpose(1, 2, 0).reshape(2, 128, KX, 2, 128)
        .reshape(2, 128, KX * 2 * 128)).astype(bf16)
    ah = np.zeros((128, 16), np.float32)
    ah[:, :8] = A.reshape(8, 128).T
    ah[:, 8:] = SC_B * h0.reshape(8, 128).T
    return {"xt": xt, "x8": x8, "bw": bw, "cw": cw, "mw": mw, "ah": ah,
            "z8": np.zeros((128, 2, HPAD), f8)}


LAST_RESULT = None


def kernel(inputs, h0, A, B, C, M):
    global LAST_RESULT
    from concourse.bass_utils import run_bass_kernel_spmd

    inputs = np.asarray(inputs, np.float32)
    h0 = np.asarray(h0, np.float32)
    A = np.asarray(A, np.float32)
    B = np.asarray(B, np.float32)
    C = np.asarray(C, np.float32)
    M = np.asarray(M, np.float32)

    if "nc" not in _CACHED:
        _CACHED["nc"] = _build_nc()
    nc = _CACHED["nc"]

    in_maps = [_prep_core_inputs(inputs, h0, A, B, C, M, c)
               for c in range(N_CORES)]
    res = run_bass_kernel_spmd(nc, in_maps, list(range(N_CORES)))
    LAST_RESULT = res
    # res: [b, och, 128o, T] per core -> [b, T, O]
    out = np.concatenate(
        [res.results[c]["out"].transpose(0, 3, 1, 2).reshape(
            B_PER_CORE, SEQ, O) for c in range(N_CORES)], axis=0)
    return np.ascontiguousarray(out, np.float32)


# revision 15
# speedup vs baseline: 1.7230x; 1.0986x over previous
"""LDS (diagonal linear state space + AR) kernel for 8 Trainium2 cores.

Computation (per batch b):
    uB[t, s]   = sum_d x[t, d] * B[d, s]
    h[t]       = A * h[t-1] + uB[t]          (h[-1] = h0, A diagonal)
    lds[t, o]  = sum_s h[t, s] * C[s, o]
    out[t, o]  = sum_{i<10} sum_d M[o, d, i] * x[t-i, d]  +  lds[t+10, o]

Sharding: data-parallel over batch, 2 batches per core, no collectives.

v4: the AR term dominates the output (~100x the lds term), so the AR
matmuls run in bf16 while the entire lds branch (uB, C) runs in fp8
DoubleRow mode (K=256 per matmul, 2x PE throughput). Scales are exact
powers of two folded through the linear recurrence: B*32 and x in fp8
give uB' = 32*uB; the f32 scan carries h' = 32*h and writes fp8 h8;
C*256 in fp8 gives lds' = 8192*lds; the AR weights M*8192 in bf16 put
both terms at the same scale in one PSUM tile, descaled by the final
scaled-copy. End-to-end error stays ~2.5e-3 (vs 2e-2 tolerance).

  - x lives twice in SBUF: bf16 [dch, 128d, PAD+T] for AR, and fp8
    DR-packed [128, 2, PAD+T] for uB; both DMA'd in t-slabs
  - uB: one DoubleRow matmul per (s-chunk, 512-col chunk) into a PSUM
    bank; 512-col f32 scans chain 8-apart (no DVE bubbles), writing
    fp8 h8 tiles [128, 2, T] packed for the C DoubleRow matmuls
  - output tiles [128o, 512t]: 20 bf16 AR matmuls (stationary M taps)
    + 4 fp8-DR C matmuls accumulate; AR matmuls depend only on x and
    statically fill the PE gaps while scans drain; C matmuls are gated
    per scan-chunk
  - out written to HBM as [b, och, 128o, T]; host transposes for free
"""

import sys

if "/opt/trn_rl_repo" not in sys.path:
    sys.path.insert(0, "/opt/trn_rl_repo")

import ml_dtypes
import numpy as np

import concourse.bass as bass
import concourse.mybir as mybir
from concourse.tile import TileContext

BSZ = 16
SEQ = 2048
D = 256  # input dim
S = 1024  # state dim
O = 256  # output dim
KX = 10
N_CORES = 8
B_PER_CORE = BSZ // N_CORES  # 2

PAD = 16  # left zero-pad on time for the AR taps (needs >= KX-1 = 9)
HPAD = 16  # right zero-pad on h time for the +10 shift (needs >= KX)
TCH = 512  # matmul stream width / chunk width (= 1 PSUM bank of f32)
NCP = SEQ // TCH  # 4 chunks per batch
FILL_MMS = 18  # matmuls' worth of fill emitted per uB half-round

SC_B = 32.0  # B scale -> h' = 32*h (|h'| < ~150, fp8 max 240)
SC_C = 256.0  # C scale -> lds' = 8192*lds
SC_OUT = SC_B * SC_C  # = 8192; M scaled by this in bf16 (exact)

F32 = mybir.dt.float32
BF16 = mybir.dt.bfloat16
F8 = mybir.dt.float8e4
DR = mybir.MatmulPerfMode.DoubleRow

_CACHED = {}


def _build_nc():
    nc = bass.Bass()

    xt_d = nc.dram_tensor("xt", [B_PER_CORE, 2, 128, PAD + SEQ], BF16,
                          kind="ExternalInput")
    x8_d = nc.dram_tensor("x8", [B_PER_CORE, 128, 2, PAD + SEQ], F8,
                          kind="ExternalInput")
    bw_d = nc.dram_tensor("bw", [128, 8, 2, 128], F8, kind="ExternalInput")
    cw_d = nc.dram_tensor("cw", [128, 4, 2, 2, 128], F8,
                          kind="ExternalInput")
    mw_d = nc.dram_tensor("mw", [2, 128, KX * 2 * 128], BF16,
                          kind="ExternalInput")
    ah_d = nc.dram_tensor("ah", [128, 16], F32, kind="ExternalInput")
    z8_d = nc.dram_tensor("z8", [128, 2, HPAD], F8, kind="ExternalInput")
    out_d = nc.dram_tensor("out", [B_PER_CORE, 2, 128, SEQ], F32,
                           kind="ExternalOutput")

    # x DMA slabs: slab k covers chunk k's reads (and AR windows)
    slabs = [(0, PAD + TCH)]
    c = PAD + TCH
    while c < PAD + SEQ:
        e = min(c + TCH, PAD + SEQ)
        slabs.append((c, e))
        c = e

    with TileContext(nc) as tc:
        with tc.tile_pool(name="persist", bufs=1) as persist, \
             tc.tile_pool(name="h8p", bufs=8) as h8_pool, \
             tc.tile_pool(name="outsb", bufs=6) as out_sbuf, \
             tc.tile_pool(name="ubps", bufs=4, space="PSUM") as ub_psum, \
             tc.tile_pool(name="outps", bufs=4, space="PSUM") as out_psum:

            # ---- persistent operands; emission order = DMA priority.
            # Few, large, need-ordered transfers: the sync queue issues
            # descriptors serially (~0.5us each), so DMA count is latency.
            bw8 = persist.tile([128, 8, 2, 128], F8, tag="bw8")
            nc.sync.dma_start(out=bw8[:], in_=bw_d[:])
            x8t = {}
            for b in range(B_PER_CORE):
                x8t[b] = persist.tile([128, 2, PAD + SEQ], F8,
                                      name=f"x8{b}", tag=f"x8{b}")
            xtt = {}
            for b in range(B_PER_CORE):
                for dch in range(2):
                    xtt[b, dch] = persist.tile([128, PAD + SEQ], BF16,
                                               name=f"xt{b}{dch}",
                                               tag=f"xt{b}{dch}")
            c0, c1 = slabs[0]
            nc.sync.dma_start(out=x8t[0][:, :, c0:c1],
                              in_=x8_d[0][:, :, c0:c1])
            ah = persist.tile([128, 16], F32, tag="ah")
            nc.sync.dma_start(out=ah[:], in_=ah_d[:])
            mwt = {}
            for dch in range(2):  # AR fills start in round 0
                t = persist.tile([128, KX * 2 * 128], BF16, tag=f"mw{dch}")
                nc.sync.dma_start(out=t[:], in_=mw_d[dch])
                mwt[dch] = t
            m01 = slabs[1][1]  # AR tile-pair 0 reads up to col 1040
            for dch in range(2):
                nc.sync.dma_start(out=xtt[0, dch][:, :m01],
                                  in_=xt_d[0, dch][:, :m01])
            nc.sync.dma_start(out=x8t[0][:, :, c1:],
                              in_=x8_d[0][:, :, c1:])
            cw8 = persist.tile([128, 4, 2, 2, 128], F8, tag="cw8")
            nc.sync.dma_start(out=cw8[:], in_=cw_d[:])
            for dch in range(2):
                nc.sync.dma_start(out=xtt[0, dch][:, m01:],
                                  in_=xt_d[0, dch][:, m01:])
            nc.sync.dma_start(out=x8t[1][:], in_=x8_d[1][:])
            for dch in range(2):
                nc.sync.dma_start(out=xtt[1, dch][:], in_=xt_d[1, dch][:])

            h8 = {}

            # ---- output-side work units (the PE fill queue) ----
            OP = {}

            def op_tile(b, tch, och):
                key = (b, tch, och)
                first = key not in OP
                if first:
                    OP[key] = out_psum.tile([128, TCH], F32, name="op",
                                            tag="op")
                return OP[key], first

            def ar_unit(b, tchs, och, w):
                # one stationary M tap streaming 1-2 time tiles
                dch, i = divmod(w, KX)
                j = i * 2 + och
                lhs = mwt[dch][:, j * 128:(j + 1) * 128]
                for tch in tchs:
                    t0 = tch * TCH
                    op, first = op_tile(b, tch, och)
                    nc.tensor.matmul(
                        out=op[:], lhsT=lhs,
                        rhs=xtt[b, dch][:, PAD + t0 - i:PAD + t0 - i + TCH],
                        start=first, stop=False,
                    )

            def c_unit(b, tch, och):
                # lds' += h8 @ C8 over 4 DoubleRow k-groups
                t0 = tch * TCH
                op, _ = op_tile(b, tch, och)
                for q in range(4):
                    nc.tensor.matmul(
                        out=op[:],
                        lhsT=cw8[:, q, :, och],
                        rhs=h8[b, q][:, :, t0 + KX:t0 + KX + TCH],
                        start=False, stop=(q == 3),
                        perf_mode=DR,
                    )

            def out_unit(b, tch, och):
                osb = out_sbuf.tile([128, TCH], F32)
                nc.scalar.activation(out=osb[:], in_=OP[(b, tch, och)][:],
                                     func=mybir.ActivationFunctionType.Copy,
                                     scale=1.0 / SC_OUT)
                nc.sync.dma_start(out=out_d[b, och][:, tch * TCH:
                                                     (tch + 1) * TCH],
                                  in_=osb[:])
                del OP[(b, tch, och)]

            def batch_fifo(b):
                # (gate, fn): gate=(b, creq) means "after scan chunk creq
                # of batch b is emitted" (C reads h; emission order defines
                # deps). AR for tiles 2/3 runs single-tile so it can slot
                # in as soon as the t0/t1 PSUM tiles retire.
                q = []

                def creq(tch):
                    return (b, min((tch * TCH + KX + TCH - 1) // TCH,
                                   NCP - 1))

                for och in range(2):
                    for w in range(2 * KX):
                        q.append((None, 2, lambda b=b, o=och, w=w:
                                  ar_unit(b, (0, 1), o, w)))
                for och in range(2):
                    q.append((creq(0), 3, lambda b=b, o=och:
                              c_unit(b, 0, o)))
                for och in range(2):
                    q.append((creq(0), 1, lambda b=b, o=och:
                              out_unit(b, 0, o)))
                for och in range(2):
                    for w in range(2 * KX):
                        q.append((None, 1, lambda b=b, o=och, w=w:
                                  ar_unit(b, (2,), o, w)))
                for och in range(2):
                    q.append((creq(1), 3, lambda b=b, o=och:
                              c_unit(b, 1, o)))
                for och in range(2):
                    q.append((creq(1), 1, lambda b=b, o=och:
                              out_unit(b, 1, o)))
                for och in range(2):
                    for w in range(2 * KX):
                        q.append((None, 1, lambda b=b, o=och, w=w:
                                  ar_unit(b, (3,), o, w)))
                for tch in (2, 3):
                    for och in range(2):
                        q.append((creq(tch), 3, lambda b=b, t=tch, o=och:
                                  c_unit(b, t, o)))
                    for och in range(2):
                        q.append((creq(tch), 1, lambda b=b, t=tch, o=och:
                                  out_unit(b, t, o)))
                return q

            fifo = batch_fifo(0) + batch_fifo(1)
            chunks_done = {0: -1, 1: -1}
            cursor = [0]

            def fill(mm_budget):
                k = 0
                while k < mm_budget and cursor[0] < len(fifo):
                    g, cost, fn = fifo[cursor[0]]
                    if g is not None and chunks_done[g[0]] < g[1]:
                        break
                    fn()
                    cursor[0] += 1
                    k += cost

            # ---- uB + scan pipeline, AR/C matmuls filling the PE gaps ----
            for b in range(B_PER_CORE):
                for qq in range(4):
                    t = h8_pool.tile([128, 2, SEQ + HPAD], F8, name="h8",
                                     tag="h8")
                    nc.sync.dma_start(out=t[:, :, SEQ:], in_=z8_d[:])
                    h8[b, qq] = t
                for cp in range(NCP):
                    t0 = cp * TCH
                    for half in range(2):
                        ubs = {}
                        for sj in range(4):
                            sch = half * 4 + sj
                            ub = ub_psum.tile([128, TCH], F32)
                            nc.tensor.matmul(
                                out=ub[:],
                                lhsT=bw8[:, sch],
                                rhs=x8t[b][:, :, PAD + t0:PAD + t0 + TCH],
                                start=True, stop=True,
                                perf_mode=DR,
                            )
                            ubs[sch] = ub
                        for sj in range(4):
                            sch = half * 4 + sj
                            qq, jj = divmod(sch, 2)
                            init = (ah[:, 8 + sch:9 + sch] if cp == 0
                                    else h8[b, qq][:, jj, t0 - 1:t0])
                            nc.vector.tensor_tensor_scan(
                                out=h8[b, qq][:, jj, t0:t0 + TCH],
                                data0=ah[:, sch:sch + 1]
                                .broadcast_to([128, TCH]),
                                data1=ubs[sch][:],
                                initial=init,
                                op0=mybir.AluOpType.mult,
                                op1=mybir.AluOpType.add,
                            )
                        fill(FILL_MMS)
                    chunks_done[b] = cp
            fill(10 ** 9)

    # Matmult (esp. fused-LDW) supports a limited number of HW sync-wait
    # slots; split excess waits into event-semaphore chains the way
    # Bacc.compile() does.
    import bass_rust as _br
    _br.move_matmul_waits_to_ldweights(nc.m)
    _br.generate_event_semaphores(nc)

    return nc


def _prep_core_inputs(inputs, h0, A, B, C, M, core):
    """Host-side shard + layout prep for one core."""
    bf16 = ml_dtypes.bfloat16
    f8 = mybir.dt.np(mybir.dt.float8e4)
    bs = slice(core * B_PER_CORE, (core + 1) * B_PER_CORE)
    x = inputs[bs]  # [2, T, D]
    xtr = np.ascontiguousarray(x.transpose(0, 2, 1))  # [2, D, T]
    xt = np.zeros((B_PER_CORE, 2, 128, PAD + SEQ), bf16)
    xt[:, :, :, PAD:] = xtr.reshape(B_PER_CORE, 2, 128, SEQ).astype(bf16)
    # x8[b, p, j, t] = x[b, t, j*128+p]
    x8 = np.zeros((B_PER_CORE, 128, 2, PAD + SEQ), f8)
    x8[:, :, :, PAD:] = xtr.reshape(B_PER_CORE, 2, 128, SEQ).transpose(
        0, 2, 1, 3).astype(f8)

    # bw[p, sch, j, s] = 32 * B[j*128+p, sch*128+s]
    bw = np.ascontiguousarray(
        (B * SC_B).reshape(2, 128, 8, 128).transpose(1, 2, 0, 3)).astype(f8)
    # cw[p, q, j, och, o] = 256 * C[(2q+j)*128+p, och*128+o]
    cw = np.ascontiguousarray(
        (C * SC_C).reshape(4, 2, 128, 2, 128).transpose(2, 0, 1, 3, 4)
    ).astype(f8)
    # mw[dch, d, (i*2+och)*128+o] = 8192 * M[och*128+o, dch*128+d, i]
    mw = np.ascontiguousarray(
        (M * SC_OUT).transpose(1, 2, 0).reshape(2, 128, KX, 2, 128)
        .reshape(2, 128, KX * 2 * 128)).astype(bf16)
    ah = np.zeros((128, 16), np.float32)
    ah[:, :8] = A.reshape(8, 128).T
    ah[:, 8:] = SC_B * h0.reshape(8, 128).T
    return {"xt": xt, "x8": x8, "bw": bw, "cw": cw, "mw": mw, "ah": ah,
            "z8": np.zeros((128, 2, HPAD), f8)}


LAST_RESULT = None


def kernel(inputs, h0, A, B, C, M):
    global LAST_RESULT
    from concourse.bass_utils import run_bass_kernel_spmd

    inputs = np.asarray(inputs, np.float32)
    h0 = np.asarray(h0, np.float32)
    A = np.asarray(A, np.float32)
    B = np.asarray(B, np.float32)
    C = np.asarray(C, np.float32)
    M = np.asarray(M, np.float32)

    if "nc" not in _CACHED:
        _CACHED["nc"] = _build_nc()
    nc = _CACHED["nc"]

    in_maps = [_prep_core_inputs(inputs, h0, A, B, C, M, c)
               for c in range(N_CORES)]
    res = run_bass_kernel_spmd(nc, in_maps, list(range(N_CORES)))
    LAST_RESULT = res
    # res: [b, och, 128o, T] per core -> [b, T, O]
    out = np.concatenate(
        [res.results[c]["out"].transpose(0, 3, 1, 2).reshape(
            B_PER_CORE, SEQ, O) for c in range(N_CORES)], axis=0)
    return np.ascontiguousarray(out, np.float32)
